# revision 6
# baseline (speedup 1.0000x reference)
"""GroupRouter Trainium2 Bass kernel.

Self-contained: hardcodes B=32768, D=768, H=256, G=8, 8 cores data-parallel
over batch. Host pre-transposes the x shard ([rows,D] -> [D,rows]) so all
contractions have their reduction dim on SBUF partitions with zero on-chip
input transposes. The dim-importance branch is computed fully transposed
(g1.T straight out of the matmul); the router branch is row-major so the
LayerNorm stats are per-partition scalars. GELU(exact erf) is built from the
ACT Erf table (same table set as Sigmoid -> no ACT table thrashing) with the
x0.5 folded into the next layer's weights (exact: power-of-two scale).
"""
import numpy as np
from contextlib import ExitStack

import concourse.bass as bass
import concourse.bacc as bacc
import concourse.mybir as mybir
import concourse.tile as tile
from concourse.bass_utils import run_bass_kernel_spmd

FP32 = mybir.dt.float32
I32 = mybir.dt.int32
AF = mybir.ActivationFunctionType
OP = mybir.AluOpType

B, D, H, G = 32768, 768, 256, 8
NC = 8
RPC = B // NC          # rows per core = 4096
ST = 512               # rows per super-tile
NST = RPC // ST        # 8 super-tiles
NSUB = ST // 128       # 4 sub-tiles of 128 rows
KD = D // 128          # 6 contraction chunks over D
INV_SQRT2 = float(np.float32(0.7071067811865476))
MAGIC = 0x1FBD1DF5     # sqrt bit-trick seed constant


def build(flags):
    nc = bacc.Bacc("TRN2", target_bir_lowering=False, debug=False, num_devices=NC)

    xT = nc.declare_dram_parameter("xT", [D, RPC], FP32, isOutput=False)
    w1 = nc.declare_dram_parameter("w1", [D, H], FP32, isOutput=False)
    dw1 = nc.declare_dram_parameter("dw1", [D, H], FP32, isOutput=False)
    w2h = nc.declare_dram_parameter("w2h", [H, 128], FP32, isOutput=False)
    w3h = nc.declare_dram_parameter("w3h", [128, G], FP32, isOutput=False)
    dw2h = nc.declare_dram_parameter("dw2h", [H, D], FP32, isOutput=False)
    db2r = nc.declare_dram_parameter("db2r", [1, D], FP32, isOutput=False)
    bgr = nc.declare_dram_parameter("bgr", [128, G], FP32, isOutput=False)
    mmask = nc.declare_dram_parameter("mmask", [128, G * G], FP32, isOutput=False)
    ident = nc.declare_dram_parameter("ident", [128, 128], FP32, isOutput=False)
    ones1 = nc.declare_dram_parameter("ones1", [1, 128], FP32, isOutput=False)
    onesP = nc.declare_dram_parameter("onesP", [128, 1], FP32, isOutput=False)
    if not flags["z_b1"]:
        b1r = nc.declare_dram_parameter("b1r", [128, H], FP32, isOutput=False)
    if not flags["id_ln"]:
        lngr = nc.declare_dram_parameter("lngr", [128, H], FP32, isOutput=False)
        lnbr = nc.declare_dram_parameter("lnbr", [128, H], FP32, isOutput=False)
    if not flags["z_b2"]:
        b2r = nc.declare_dram_parameter("b2r", [128, 128], FP32, isOutput=False)
    if not flags["z_db1"]:
        db1t = nc.declare_dram_parameter("db1t", [128, 2], FP32, isOutput=False)
        db1ts = nc.declare_dram_parameter("db1ts", [128, 2], FP32, isOutput=False)

    mask_o = nc.declare_dram_parameter("mask_o", [RPC, D], FP32, isOutput=True)
    gm_o = nc.declare_dram_parameter("gm_o", [RPC, D], FP32, isOutput=True)
    dw_o = nc.declare_dram_parameter("dw_o", [RPC, D], FP32, isOutput=True)
    probs_o = nc.declare_dram_parameter("probs_o", [RPC, G], FP32, isOutput=True)
    sel_o = nc.declare_dram_parameter("sel_o", [RPC, G], FP32, isOutput=True)
    stats_o = nc.declare_dram_parameter("stats_o", [1, 64], FP32, isOutput=True)

    xv = xT[:].rearrange("(k p) n -> p k n", p=128)            # [128, 6, RPC]
    w1v = w1[:].rearrange("(k p) h -> p k h", p=128)           # [128, 6, 256]
    dw1v = dw1[:].rearrange("(k p) (m h) -> p k m h", p=128, m=2)  # [128,6,2,128]
    w2v = w2h[:].rearrange("(m p) h -> p m h", p=128)          # [128, 2, 128]
    dw2v = dw2h[:].rearrange("(m p) d -> p m d", p=128)        # [128, 2, 768]
    maskv = mask_o[:].rearrange("(a s p) d -> a p s d", s=NSUB, p=128)
    gmv = gm_o[:].rearrange("(a s p) d -> a p s d", s=NSUB, p=128)
    dwv = dw_o[:].rearrange("(a s p) d -> a p s d", s=NSUB, p=128)
    probsv = probs_o[:].rearrange("(a s p) g -> a p s g", s=NSUB, p=128)
    selv = sel_o[:].rearrange("(a s p) g -> a p s g", s=NSUB, p=128)

    with tile.TileContext(nc) as tc, \
         tc.tile_pool(name="consts", bufs=1) as cst, \
         tc.tile_pool(name="xin", bufs=2) as xin, \
         tc.tile_pool(name="big", bufs=2) as big, \
         tc.tile_pool(name="mid", bufs=3) as mid, \
         tc.tile_pool(name="ypool", bufs=6) as ypool, \
         tc.tile_pool(name="small", bufs=2) as sml, \
         tc.tile_pool(name="ps_h1", bufs=2, space="PSUM") as ps_h1p, \
         tc.tile_pool(name="ps_g1", bufs=1, space="PSUM") as ps_g1p, \
         tc.tile_pool(name="ps_dim", bufs=1, space="PSUM") as ps_dimp, \
         tc.tile_pool(name="ps_sm", bufs=1, space="PSUM") as ps_smp:

        # ---- constants into SBUF (one time) ----
        w1_sb = cst.tile([128, KD, H], FP32)
        nc.sync.dma_start(w1_sb[:], w1v)
        dw1_sb = cst.tile([128, KD, 2, 128], FP32)
        nc.sync.dma_start(dw1_sb[:], dw1v)
        w2_sb = cst.tile([128, 2, 128], FP32)
        nc.sync.dma_start(w2_sb[:], w2v)
        w3_sb = cst.tile([128, G], FP32)
        nc.sync.dma_start(w3_sb[:], w3h[:])
        dw2_sb = cst.tile([128, 2, D], FP32)
        nc.sync.dma_start(dw2_sb[:], dw2v)
        db2_sb = cst.tile([1, D], FP32)
        nc.sync.dma_start(db2_sb[:], db2r[:])
        bgr_sb = cst.tile([128, G], FP32)
        nc.sync.dma_start(bgr_sb[:], bgr[:])
        mm_sb = cst.tile([128, G * G], FP32)
        nc.sync.dma_start(mm_sb[:], mmask[:])
        id_sb = cst.tile([128, 128], FP32)
        nc.sync.dma_start(id_sb[:], ident[:])
        ones1_sb = cst.tile([1, 128], FP32)
        nc.sync.dma_start(ones1_sb[:], ones1[:])
        onesP_sb = cst.tile([128, 1], FP32)
        nc.sync.dma_start(onesP_sb[:], onesP[:])
        if not flags["z_b1"]:
            b1_sb = cst.tile([128, H], FP32)
            nc.sync.dma_start(b1_sb[:], b1r[:])
        if not flags["id_ln"]:
            lng_sb = cst.tile([128, H], FP32)
            nc.sync.dma_start(lng_sb[:], lngr[:])
            lnb_sb = cst.tile([128, H], FP32)
            nc.sync.dma_start(lnb_sb[:], lnbr[:])
        if not flags["z_b2"]:
            b2_sb = cst.tile([128, 128], FP32)
            nc.sync.dma_start(b2_sb[:], b2r[:])
        if not flags["z_db1"]:
            db1_sb = cst.tile([128, 2], FP32)
            nc.sync.dma_start(db1_sb[:], db1t[:])
            db1s_sb = cst.tile([128, 2], FP32)
            nc.sync.dma_start(db1s_sb[:], db1ts[:])

        acc_p = cst.tile([128, NSUB * G], FP32)
        nc.gpsimd.memset(acc_p[:], 0.0)
        acc_g = cst.tile([128, NSUB * G], FP32)
        nc.gpsimd.memset(acc_g[:], 0.0)

        mmb = mm_sb[:].rearrange("p (a b) -> p a b", a=G).unsqueeze(1) \
                      .broadcast_to([128, NSUB, G, G])

        for st in range(NST):
            r0 = st * ST
            xts = xin.tile([128, KD, ST], FP32, tag="xts")
            nc.sync.dma_start(xts[:], xv[:, :, r0:r0 + ST])

            # ---- dim branch, transposed: g1T = 2*gelu(dw1.T @ x.T) ----
            pg = ps_g1p.tile([128, 2, ST], FP32, tag="pg")
            for m in range(2):
                for k in range(KD):
                    nc.tensor.matmul(pg[:, m, :], dw1_sb[:, k, m, :], xts[:, k, :],
                                     start=(k == 0), stop=(k == KD - 1))
            g1u_ap = pg[:]
            if not flags["z_db1"]:
                g1u = mid.tile([128, 2, ST], FP32, tag="g1u")
                for m in range(2):
                    nc.vector.tensor_scalar(g1u[:, m, :], pg[:, m, :],
                                            db1_sb[:, m:m + 1], None, OP.add)
                g1u_ap = g1u[:]
            g1e = mid.tile([128, 2, ST], FP32, tag="g1e")
            if flags["z_db1"]:
                nc.scalar.activation(
                    g1e[:].rearrange("p m n -> p (m n)"),
                    g1u_ap.rearrange("p m n -> p (m n)"), AF.Erf, scale=INV_SQRT2)
            else:
                for m in range(2):
                    nc.scalar.activation(g1e[:, m, :], pg[:, m, :], AF.Erf,
                                         bias=db1s_sb[:, m:m + 1], scale=INV_SQRT2)
            g1T = mid.tile([128, 2, ST], FP32, tag="g1T")
            nc.vector.scalar_tensor_tensor(
                g1T[:].rearrange("p m n -> p (m n)"),
                g1e[:].rearrange("p m n -> p (m n)"), 1.0,
                g1u_ap.rearrange("p m n -> p (m n)"), OP.add, OP.mult)

            # ---- per-ST stat tiles ----
            negmu = sml.tile([128, NSUB], FP32, tag="negmu")
            sumsq = sml.tile([128, NSUB], FP32, tag="sumsq")
            sums = sml.tile([128, NSUB], FP32, tag="sums")
            lg32 = sml.tile([128, NSUB, G], FP32, tag="lg32")
            dwq = big.tile([128, NSUB, D], FP32, tag="dwq")
            gmq = big.tile([128, NSUB, D], FP32, tag="gmq")
            mkq = big.tile([128, NSUB, D], FP32, tag="mkq")

            y_l = []
            for s in range(NSUB):
                c0 = s * 128
                # ---- router h1pre row-major: [128 rows, 256] ----
                ph1 = ps_h1p.tile([128, H], FP32, tag="ph1")
                for k in range(KD):
                    nc.tensor.matmul(ph1[:], xts[:, k, c0:c0 + 128], w1_sb[:, k, :],
                                     start=(k == 0), stop=(k == KD - 1))
                if not flags["z_b1"]:
                    nc.vector.tensor_tensor(ph1[:], ph1[:], b1_sb[:], OP.add)
                # copy PSUM->SBUF with fused row-sum; Square gives sum of squares
                y = ypool.tile([128, H], FP32, tag="y")
                y_l.append(y)
                nc.scalar.activation(y[:], ph1[:], AF.Identity,
                                     accum_out=sums[:, s:s + 1])
                sqs = mid.tile([128, H], FP32, tag="sqs")
                nc.scalar.activation(sqs[:], ph1[:], AF.Square,
                                     accum_out=sumsq[:, s:s + 1])

                # ---- dim branch out: dimpre = g1T.T @ dw2h + db2 ----
                pd = ps_dimp.tile([128, 2, 512], FP32, tag="pd")
                for half in range(2):
                    dcol = half * 384
                    for m in range(2):
                        nc.tensor.matmul(pd[:, half, 0:384],
                                         g1T[:, m, c0:c0 + 128],
                                         dw2_sb[:, m, dcol:dcol + 384],
                                         start=(m == 0), stop=False)
                    nc.tensor.matmul(pd[:, half, 0:384], ones1_sb[:],
                                     db2_sb[:, dcol:dcol + 384],
                                     start=False, stop=True)
                nc.scalar.activation(
                    dwq[:, s, :].rearrange("p (m d) -> p m d", m=2),
                    pd[:, :, 0:384], AF.Sigmoid)

            # ---- LN rsqrt via reciprocal + sqrt bit-trick + 3 Newton ----
            nc.vector.tensor_scalar(negmu[:], sums[:], -1.0 / H, None, OP.mult)
            m2 = sml.tile([128, NSUB], FP32, tag="m2")
            nc.vector.tensor_tensor(m2[:], negmu[:], negmu[:], OP.mult)
            varep = sml.tile([128, NSUB], FP32, tag="varep")
            nc.vector.scalar_tensor_tensor(varep[:], sumsq[:], 1.0 / H, m2[:],
                                           OP.mult, OP.subtract)
            nc.vector.tensor_scalar(varep[:], varep[:], 1e-5, None, OP.add)
            rcp = sml.tile([128, NSUB], FP32, tag="rcp")
            nc.vector.reciprocal(rcp[:], varep[:])
            ki = sml.tile([128, NSUB], I32, tag="ki")
            nc.vector.tensor_scalar(ki[:], rcp[:].bitcast(I32), 1, None,
                                    OP.arith_shift_right)
            nc.vector.tensor_scalar(ki[:], ki[:], MAGIC, None, OP.add)
            rs = sml.tile([128, NSUB], FP32, tag="rs")
            nc.vector.tensor_copy(rs[:], ki[:].bitcast(FP32))
            nt = sml.tile([128, NSUB], FP32, tag="nt")
            for _ in range(3):
                nc.vector.tensor_tensor(nt[:], rs[:], rs[:], OP.mult)
                nc.vector.tensor_tensor(nt[:], varep[:], nt[:], OP.mult)
                nc.vector.tensor_scalar(nt[:], nt[:], -0.5, 1.5, OP.mult, OP.add)
                nc.vector.tensor_tensor(rs[:], rs[:], nt[:], OP.mult)

            for s in range(NSUB):
                c0 = s * 128
                # u = (y - mu) * rs ; h1d = (erf(u/sqrt2)+1)*u  (=2*gelu(u))
                u = mid.tile([128, H], FP32, tag="u")
                nc.vector.tensor_scalar(u[:], y_l[s][:], negmu[:, s:s + 1],
                                        rs[:, s:s + 1], OP.add, OP.mult)
                u_ap = u[:]
                if not flags["id_ln"]:
                    u2 = mid.tile([128, H], FP32, tag="u2")
                    nc.vector.tensor_tensor(u2[:], u[:], lng_sb[:], OP.mult)
                    nc.vector.tensor_tensor(u2[:], u2[:], lnb_sb[:], OP.add)
                    u_ap = u2[:]
                he = mid.tile([128, H], FP32, tag="he")
                nc.scalar.activation(he[:], u_ap, AF.Erf, scale=INV_SQRT2)
                h1d = mid.tile([128, H], FP32, tag="h1d")
                nc.vector.scalar_tensor_tensor(h1d[:], he[:], 1.0, u_ap,
                                               OP.add, OP.mult)
                # h1T (2 PE transposes + copies)
                h1T = mid.tile([128, 2, 128], FP32, tag="h1T")
                for m in range(2):
                    ptr = ps_smp.tile([128, 128], FP32, tag="ptr")
                    nc.tensor.matmul(ptr[:], h1d[:, m * 128:(m + 1) * 128],
                                     id_sb[:], is_transpose=True)
                    nc.vector.tensor_copy(h1T[:, m, :], ptr[:])
                # h2 = 2*gelu(h1 @ w2h*?)  (w2h carries the x0.5)
                ph2 = ps_smp.tile([128, 128], FP32, tag="ph2")
                for m in range(2):
                    nc.tensor.matmul(ph2[:], h1T[:, m, :], w2_sb[:, m, :],
                                     start=(m == 0), stop=(m == 1))
                if not flags["z_b2"]:
                    nc.vector.tensor_tensor(ph2[:], ph2[:], b2_sb[:], OP.add)
                e2 = mid.tile([128, 128], FP32, tag="e2")
                nc.scalar.activation(e2[:], ph2[:], AF.Erf, scale=INV_SQRT2)
                h2d = mid.tile([128, 128], FP32, tag="h2d")
                nc.vector.scalar_tensor_tensor(h2d[:], e2[:], 1.0, ph2[:],
                                               OP.add, OP.mult)
                ptr2 = ps_smp.tile([128, 128], FP32, tag="ptr")
                nc.tensor.matmul(ptr2[:], h2d[:], id_sb[:], is_transpose=True)
                h2T = mid.tile([128, 128], FP32, tag="h2T")
                nc.vector.tensor_copy(h2T[:], ptr2[:])
                plg = ps_smp.tile([128, G], FP32, tag="ph2")
                nc.tensor.matmul(plg[:], h2T[:], w3_sb[:], start=True, stop=True)
                nc.vector.tensor_tensor(lg32[:, s, :], plg[:], bgr_sb[:], OP.add)

            # ---- selection: rank from logits, pairwise compares ----
            probs_st = sml.tile([128, NSUB, G], FP32, tag="probs_st")
            nc.scalar.activation(probs_st[:].rearrange("p s g -> p (s g)"),
                                 lg32[:].rearrange("p s g -> p (s g)"), AF.Sigmoid)
            a_i = lg32[:].unsqueeze(3).broadcast_to([128, NSUB, G, G])
            a_j = lg32[:].unsqueeze(2).broadcast_to([128, NSUB, G, G])
            gtt = sml.tile([128, NSUB, G, G], FP32, tag="gtt")
            nc.vector.tensor_tensor(gtt[:], a_j, a_i, OP.is_gt)
            eqt = sml.tile([128, NSUB, G, G], FP32, tag="eqt")
            nc.vector.tensor_tensor(eqt[:], a_j, a_i, OP.is_equal)
            nc.vector.tensor_tensor(eqt[:], eqt[:], mmb, OP.mult)
            nc.vector.tensor_tensor(gtt[:], gtt[:], eqt[:], OP.add)
            rank = sml.tile([128, NSUB, G], FP32, tag="rank")
            nc.vector.tensor_reduce(rank[:], gtt[:], mybir.AxisListType.X, OP.add)
            lt2 = sml.tile([128, NSUB, G], FP32, tag="lt2")
            nc.vector.tensor_scalar(lt2[:], rank[:], 2.0, None, OP.is_lt)
            lt6 = sml.tile([128, NSUB, G], FP32, tag="lt6")
            nc.vector.tensor_scalar(lt6[:], rank[:], 6.0, None, OP.is_lt)
            gt0 = sml.tile([128, NSUB, G], FP32, tag="gt0")
            nc.vector.tensor_scalar(gt0[:], lg32[:], 0.0, None, OP.is_gt)
            nc.vector.tensor_tensor(lt6[:], lt6[:], lt2[:], OP.subtract)
            nc.vector.tensor_tensor(lt6[:], lt6[:], gt0[:], OP.mult)
            hard = sml.tile([128, NSUB, G], FP32, tag="hard")
            nc.vector.tensor_tensor(hard[:], lt2[:], lt6[:], OP.add)
            sel_st = sml.tile([128, NSUB, G], FP32, tag="sel_st")
            nc.vector.tensor_tensor(sel_st[:], hard[:], probs_st[:], OP.subtract)
            nc.vector.tensor_tensor(sel_st[:], sel_st[:], probs_st[:], OP.add)

            nc.vector.tensor_tensor(acc_p[:], acc_p[:],
                                    probs_st[:].rearrange("p s g -> p (s g)"), OP.add)
            nc.vector.tensor_tensor(acc_g[:], acc_g[:],
                                    gt0[:].rearrange("p s g -> p (s g)"), OP.add)

            # ---- expand + mask (pool engine) ----
            for s in range(NSUB):
                selb = sel_st[:, s, :].unsqueeze(2).broadcast_to([128, G, D // G])
                nc.gpsimd.tensor_copy(
                    gmq[:, s, :].rearrange("p (g d) -> p g d", g=G), selb)
                nc.gpsimd.tensor_tensor(
                    mkq[:, s, :].rearrange("p (g d) -> p g d", g=G), selb,
                    dwq[:, s, :].rearrange("p (g d) -> p g d", g=G), OP.mult)

            nc.sync.dma_start(dwv[st], dwq[:])
            nc.sync.dma_start(gmv[st], gmq[:])
            nc.sync.dma_start(maskv[st], mkq[:])
            nc.sync.dma_start(probsv[st], probs_st[:])
            nc.sync.dma_start(selv[st], sel_st[:])

        # ---- final partition reduction of stats via PE ----
        pst = ps_smp.tile([1, 64], FP32, tag="ph2")
        nc.tensor.matmul(pst[:, 0:32], onesP_sb[:], acc_p[:], start=True, stop=True)
        nc.tensor.matmul(pst[:, 32:64], onesP_sb[:], acc_g[:], start=True, stop=True)
        stat_sb = cst.tile([1, 64], FP32)
        nc.vector.tensor_copy(stat_sb[:], pst[:])
        nc.sync.dma_start(stats_o[:], stat_sb[:])

    nc.compile()
    return nc


_CACHE = {}


def _get_nc(flags):
    key = tuple(sorted(flags.items()))
    if key not in _CACHE:
        _CACHE[key] = build(flags)
    return _CACHE[key]


def _prep(query_embedding, w1, b1, ln_g, ln_b, w2, b2, w3, b3,
          dw1, db1, dw2, db2, group_importance):
    f32 = np.float32
    asf = lambda a: np.ascontiguousarray(np.asarray(a, f32))
    flags = {
        "z_b1": bool(np.all(np.asarray(b1) == 0)),
        "id_ln": bool(np.all(np.asarray(ln_g) == 1) and np.all(np.asarray(ln_b) == 0)),
        "z_b2": bool(np.all(np.asarray(b2) == 0)),
        "z_db1": bool(np.all(np.asarray(db1) == 0)),
    }
    common = {
        "w1": asf(w1),
        "dw1": asf(dw1),
        "w2h": asf(np.asarray(w2, f32) * f32(0.5)),
        "w3h": asf(np.asarray(w3, f32) * f32(0.5)),
        "dw2h": asf(np.asarray(dw2, f32) * f32(0.5)),
        "db2r": asf(np.asarray(db2, f32).reshape(1, D)),
        "bgr": asf(np.broadcast_to(np.asarray(b3, f32) + np.asarray(group_importance, f32), (128, G))),
        "mmask": asf(np.broadcast_to(
            (np.arange(G)[None, :] < np.arange(G)[:, None]).astype(f32).reshape(1, G * G),
            (128, G * G))),
        "ident": np.eye(128, dtype=f32),
        "ones1": np.ones((1, 128), f32),
        "onesP": np.ones((128, 1), f32),
    }
    if not flags["z_b1"]:
        common["b1r"] = asf(np.broadcast_to(np.asarray(b1, f32), (128, H)))
    if not flags["id_ln"]:
        common["lngr"] = asf(np.broadcast_to(np.asarray(ln_g, f32), (128, H)))
        common["lnbr"] = asf(np.broadcast_to(np.asarray(ln_b, f32), (128, H)))
    if not flags["z_b2"]:
        common["b2r"] = asf(np.broadcast_to(np.asarray(b2, f32), (128, 128)))
    if not flags["z_db1"]:
        d = np.asarray(db1, f32).reshape(2, 128).T          # [128, 2]
        common["db1t"] = asf(d)
        common["db1ts"] = asf(d * f32(INV_SQRT2))
    x = np.asarray(query_embedding, f32)
    in_maps = []
    for c in range(NC):
        m = dict(common)
        m["xT"] = np.ascontiguousarray(x[c * RPC:(c + 1) * RPC, :].T)
        in_maps.append(m)
    return flags, in_maps


def _postprocess(results):
    f32 = np.float32
    mask = np.concatenate([r["mask_o"] for r in results], 0)
    gm = np.concatenate([r["gm_o"] for r in results], 0)
    dw = np.concatenate([r["dw_o"] for r in results], 0)
    probs = np.concatenate([r["probs_o"] for r in results], 0)
    sel = np.concatenate([r["sel_o"] for r in results], 0)
    sum_p = np.zeros(G, np.float64)
    sum_g = np.zeros(G, np.float64)
    for r in results:
        st = np.asarray(r["stats_o"], np.float64).reshape(64)
        sum_p += st[0:32].reshape(NSUB, G).sum(0)
        sum_g += st[32:64].reshape(NSUB, G).sum(0)
    active_groups = f32(sum_g.sum() / B)
    group_usage = (sum_g / B).astype(f32)
    load_balance = f32(G * np.sum((sum_p / B) * (1.0 / G)))
    active_dims = f32(np.count_nonzero(mask > 0.5) / B)
    return (mask, gm, dw, probs, sel, active_groups, group_usage,
            load_balance, active_dims)


def _run(inputs, trace=False):
    flags, in_maps = _prep(**inputs)
    nc = _get_nc(flags)
    if trace:
        _install_profshim()
    res = run_bass_kernel_spmd(nc, in_maps, core_ids=list(range(NC)), trace=trace)
    return _postprocess(res.results), res.exec_time_ns


def kernel(**inputs):
    out, _ = _run(inputs, trace=False)
    return out


def kernel_traced(**inputs):
    return _run(inputs, trace=True)


def _install_profshim():
    import sys, types
    if "antenv.axon_hooks" in sys.modules:
        return
    import antenv
    mod = types.ModuleType("antenv.axon_hooks")
    _hook = [None]
    mod.set_axon_ntff_profile_hook = lambda h: _hook.__setitem__(0, h)
    mod.get_axon_ntff_profile_hook = lambda: _hook[0]
    sys.modules["antenv.axon_hooks"] = mod
    antenv.axon_hooks = mod
    from trn_agent_boot.trn_boot import _ntff_profile_via_ctypes
    mod.set_axon_ntff_profile_hook(_ntff_profile_via_ctypes("/opt/axon/libaxon_pjrt.so"))
